# revision 21
# baseline (speedup 1.0000x reference)
"""Trainium2 Bass kernel for BoundaryPredictor2 (B=4, L=1024, D=512, H=8).

Sharding: 8 cores = 4 batch rows x 2 token-halves (512 tokens each).

Phase C-first (feature-major activations throughout, zero PE transposes):
  l2norm -> layernorm -> z' matmul + fused scores (host-precomputed
  Wpk^T@q and Wpo@Wpv) -> z' = (hn@Weff^T)*e; ONE merged pairwise
  AllGather of z' (532KB) fires early and hides under the boundary chain.
Phase A (boundary chain, fp32r matmuls; cos reduction in true fp32):
  MLP(gelu) -> residual l2norm -> qh/kh -> adjacent cos -> hard.
  The cross-half "straddle" pair ships through the hard-AllGather and is
  patched into the chunk offsets only (it provably can't change any
  other seg id).
Phase B: seg ids entirely in [128, 8] token-chunk layout: strictly-upper
  triangular matmul gives the within-chunk exclusive prefix, chunk
  totals via ones-reduction, an 8-wide scan for chunk offsets, both
  accumulated in one PSUM bank. No [1, 1024] scans, no DRAM round trip.
Pooling: one-hot A blocks x z' -> own 512 segment slots in PSUM
  (Wpo pre-folded, so PSUM rows are final up to 1/denom), normalize,
  store [512, 512].

The program is uniform SPMD; all per-core differences (token slice,
masks, iota offset) are host-fed data. A dummy warmup collective
triggers at ~2us to absorb the CC engine's ~40us init latency.
"""

from contextlib import ExitStack

import numpy as np

import concourse.bass as bass
import concourse.tile as tile
from concourse import bacc, mybir
from concourse.bass_utils import run_bass_kernel_spmd

FP = mybir.dt.float32
FR = mybir.dt.float32r
F16 = mybir.dt.float16
AF = mybir.ActivationFunctionType
OP = mybir.AluOpType
AX = mybir.AxisListType

N_CORES = 8
B, L, D = 4, 1024, 512
H, HD = 8, 64
TPC = 512           # own tokens per core
NCH = D // 128      # feature chunks (4)
SCALE = HD ** -0.5
ZF = D + H          # z feature width (512 folded-V cols + 8 e cols)
CCW = 3 * TPC       # exchange-1 payload: hard row + kh col0 + qh col511
TG = ((0, 256), (256, 256))   # phase-A token pipeline groups

_CACHE = {}


def _emit(nc, tc, prm, out):
    ctx = ExitStack()
    cpool = ctx.enter_context(tc.tile_pool(name="consts", bufs=1))
    wpool = ctx.enter_context(tc.tile_pool(name="weights", bufs=1))
    apool = ctx.enter_context(tc.tile_pool(name="acts", bufs=1))
    spool = ctx.enter_context(tc.tile_pool(name="scratch", bufs=2))
    rpool = ctx.enter_context(tc.tile_pool(name="rows", bufs=1))
    psm = ctx.enter_context(tc.tile_pool(name="psm", bufs=3, space="PSUM"))
    pbig = ctx.enter_context(tc.tile_pool(name="pbig", bufs=4, space="PSUM"))
    dpool = ctx.enter_context(tc.tile_pool(name="dram", bufs=1, space="DRAM"))

    def ps_small():
        return psm.tile([128, 512], FP, tag="ps1", name="ps1")

    def ps_big():
        return pbig.tile([128, 512], FP, tag="pb", name="pb")

    def dma(dst, src):
        nc.sync.dma_start(out=dst, in_=src)

    def load(pool, name, shape, dt=FP, tag=None):
        t = pool.tile(list(shape), dt, tag=tag or name, name=name)
        dma(t[:], prm[name])
        return t

    # ---- warmup collective FIRST: absorbs the CC engine init latency ----
    pairs = [[2 * i, 2 * i + 1] for i in range(N_CORES // 2)]
    simb = load(cpool, "simb", (1, 1))
    wui = dpool.tile([1, 1], FP, tag="wui", name="wui")
    wuo = dpool.tile([2, 1], FP, tag="wuo", name="wuo")
    nc.sync.dma_start(out=wui[:], in_=simb[:])
    nc.gpsimd.collective_compute(
        "AllGather", OP.bypass, replica_groups=pairs,
        ins=[wui.opt()], outs=[wuo.opt()])

    # ---- input DMAs in consumption order ----
    xT = [apool.tile([128, TPC], FP, tag=f"xT{c}", name=f"xT{c}")
          for c in range(NCH)]
    for c in range(NCH):
        dma(xT[c][:], prm["hT"][c * 128:(c + 1) * 128, :])
    ones128 = load(cpool, "ones128", (128, 1))
    ones_r = load(cpool, "ones_r", (1, 128))
    ident = load(cpool, "ident", (128, 128))
    wkeff = cpool.tile([128, NCH * H], FR, tag="wkeff", name="wkeff")
    for c in range(NCH):
        dma(wkeff[:, c * H:(c + 1) * H],
            prm["wkeff"][c * 128:(c + 1) * 128, :])
    lenmask = load(cpool, "lenmask", (128, 4))
    lng = load(cpool, "lng", (128, NCH))
    lnb = load(cpool, "lnb", (128, NCH))
    bias1 = load(cpool, "bias1", (128, NCH))
    bias2 = load(cpool, "bias2", (128, NCH))

    wt = {}
    for w in ("wpvt", "w1t", "w2t", "wqt", "wkt", "wpot"):
        wdt = F16 if w == "wpot" else FR
        wt[w] = [wpool.tile([128, D], wdt, tag=f"{w}{c}", name=f"{w}{c}")
                 for c in range(NCH)]
        for c in range(NCH):
            dma(wt[w][c][:], prm[w][c * 128:(c + 1) * 128, :])
    e8 = load(cpool, "e8", (8, D))

    vm8 = load(cpool, "vm8", (128, 8))
    st8 = load(cpool, "st8", (128, 8))
    strc = load(cpool, "strc", (1, 1))
    ltri = load(cpool, "ltri", (128, 128))
    iota512 = load(cpool, "iota512", (128, TPC))

    # FR twins (fp32r matmul operands must be produced as fp32r)
    ones128r = cpool.tile([128, 1], FR, tag="ones128r", name="ones128r")
    nc.vector.tensor_copy(ones128r[:], ones128[:])
    ones_rr = cpool.tile([1, 128], FR, tag="ones_rr", name="ones_rr")
    nc.vector.tensor_copy(ones_rr[:], ones_r[:])
    identr = cpool.tile([128, 128], FR, tag="identr", name="identr")
    nc.vector.tensor_copy(identr[:], ident[:])

    def col(t, c):
        return t[:, c:c + 1]

    # 1/sqrt via ACT Sqrt (<=2ulp) + fast DVE reciprocal (~51ulp):
    # norm-scale errors are multiplicative on cos, so sign-safe.
    def rsqrt_row(dst, src, eps, mode, fr=False, w=TPC):
        sm = rpool.tile([1, TPC], FP, tag="rs_sm", name="rs_sm",
                        bufs=3)[0:1, 0:w]
        nc.vector.tensor_scalar(out=sm, in0=src, scalar1=eps,
                                scalar2=None,
                                op0=(OP.max if mode == "clip" else OP.add))
        sqv = rpool.tile([1, TPC], FP, tag="rs_sq", name="rs_sq",
                         bufs=3)[0:1, 0:w]
        nc.scalar.activation(sqv, sm, AF.Sqrt)
        if fr:
            r0 = rpool.tile([1, TPC], FP, tag="rs_r0", name="rs_r0",
                            bufs=3)[0:1, 0:w]
            nc.vector.reciprocal_approx_fast(r0, sqv)
            nc.vector.tensor_copy(dst, r0)
        else:
            nc.vector.reciprocal_approx_fast(dst, sqv)

    # dst[c] = src[c] * rsqrt(sum_d src^2); token-group pipelined
    def l2norm_fm(src_tiles, dst_tiles, msq_keep=None):
        pss = {}
        for g, (g0, gn) in enumerate(TG):
            ps = ps_small()
            for c in range(NCH):
                sq = spool.tile([128, 256], FR, tag="sq", name="sq", bufs=4)
                nc.vector.tensor_mul(sq[:], src_tiles[c][:, g0:g0 + gn],
                                     src_tiles[c][:, g0:g0 + gn])
                nc.tensor.matmul(ps[0:1, 0:gn], ones128r[:], sq[:],
                                 start=(c == 0), stop=(c == NCH - 1))
            pss[g] = ps
            r = rpool.tile([1, 256], FR, tag="nrm_r", name="nrm_r", bufs=2)
            rsqrt_row(r[0:1, :], ps[0:1, 0:gn], 1e-16, "clip", fr=True,
                      w=gn)
            rb = ps_big()
            nc.tensor.matmul(rb[:, 0:gn], ones_rr[:], r[0:1, :],
                             start=True, stop=True)
            for c in range(NCH):
                nc.vector.tensor_mul(dst_tiles[c][:, g0:g0 + gn],
                                     src_tiles[c][:, g0:g0 + gn],
                                     rb[:, 0:gn])
        if msq_keep is not None:
            nc.vector.tensor_copy(msq_keep[0:1, 0:256], pss[0][0:1, 0:256])
            nc.vector.tensor_copy(msq_keep[0:1, 256:512], pss[1][0:1, 0:256])

    # ---- l2norm of hidden (shared by boundary chain and layernorm) ----
    hn2r = [apool.tile([128, TPC], FR, tag=f"hn2r_{c}", name=f"hn2r_{c}")
            for c in range(NCH)]
    msq = rpool.tile([1, TPC], FP, tag="msq", name="msq")[0:1, :]
    l2norm_fm(xT, hn2r, msq_keep=msq)

    # ---- boundary-chain linears (emitted interleaved with LN/z so the
    # PE stays busy through the LN stats serial chain) ----
    def linear_fm(w, src_tiles, ech, g, resid=False):
        g0, gn = TG[g]
        ps = ps_big()
        for c in range(NCH):
            nc.tensor.matmul(ps[:, 0:gn],
                             wt[w][c][:, ech * 128:(ech + 1) * 128],
                             src_tiles[c][:, g0:g0 + gn],
                             start=(c == 0), stop=False if resid else
                             (c == NCH - 1))
        if resid:
            nc.tensor.matmul(ps[:, 0:gn], identr[:],
                             hn2r[ech][:, g0:g0 + gn],
                             start=False, stop=True)
        return ps[:, 0:gn]

    t1 = [apool.tile([128, TPC], FR, tag=f"t1_{c}", name=f"t1_{c}")
          for c in range(NCH)]

    def w1_pass(g):
        g0, gn = TG[g]
        for ech in range(NCH):
            ps = linear_fm("w1t", hn2r, ech, g)
            nc.scalar.activation(t1[ech][:, g0:g0 + gn], ps, AF.Gelu,
                                 bias=col(bias1, ech))

    w1_pass(0)

    # ---- layernorm -> z' + scores; merged early z AllGather ----
    mups = ps_small()
    for c in range(NCH):
        nc.tensor.matmul(mups[0:1, :], ones128[:], xT[c][:],
                         start=(c == 0), stop=(c == NCH - 1))
    mu = rpool.tile([1, TPC], FP, tag="mu", name="mu")[0:1, :]
    nc.vector.tensor_scalar(out=mu, in0=mups[0:1, :], scalar1=1.0 / D,
                            scalar2=None, op0=OP.mult)
    var = rpool.tile([1, TPC], FP, tag="var", name="var")[0:1, :]
    nc.vector.tensor_scalar(out=var, in0=msq, scalar1=1.0 / D,
                            scalar2=None, op0=OP.mult)
    mu2 = rpool.tile([1, TPC], FP, tag="mu2", name="mu2")[0:1, :]
    nc.vector.tensor_mul(mu2, mu, mu)
    nc.vector.tensor_sub(var, var, mu2)
    rstd = rpool.tile([1, TPC], FP, tag="rstd", name="rstd")[0:1, :]
    rsqrt_row(rstd, var, 1e-5, "add")
    mub = ps_big()
    nc.tensor.matmul(mub[:], ones_r[:], mu, start=True, stop=True)
    rstdb = ps_big()
    nc.tensor.matmul(rstdb[:], ones_r[:], rstd, start=True, stop=True)
    w1_pass(1)
    hn = [apool.tile([128, TPC], FR, tag=f"hn_{c}", name=f"hn_{c}")
          for c in range(NCH)]
    for c in range(NCH):
        ht = spool.tile([128, TPC], FP, tag="htmp", name="htmp")
        nc.vector.tensor_sub(ht[:], xT[c][:], mub[:])
        nc.vector.tensor_mul(ht[:], ht[:], rstdb[:])
        nc.vector.tensor_scalar(out=hn[c][:], in0=ht[:],
                                scalar1=col(lng, c), scalar2=col(lnb, c),
                                op0=OP.mult, op1=OP.add)

    z = [apool.tile([128, ZF], F16, tag=f"z_{t}", name=f"z_{t}")
         for t in range(4)]
    for tch in range(4):
        tsl = slice(tch * 128, (tch + 1) * 128)
        scps = ps_small()
        for c in range(NCH):
            nc.tensor.matmul(scps[:, 0:H], hn[c][:, tsl],
                             wkeff[:, c * H:(c + 1) * H],
                             start=(c == 0), stop=(c == NCH - 1))
        e = spool.tile([128, H], FP, tag="e", name="e")
        nc.scalar.activation(e[:], scps[:, 0:H], AF.Exp, scale=SCALE)
        nc.vector.tensor_scalar(out=e[:], in0=e[:],
                                scalar1=lenmask[:, tch:tch + 1], scalar2=None,
                                op0=OP.mult)
        vp = ps_big()
        for c in range(NCH):
            nc.tensor.matmul(vp[:], hn[c][:, tsl], wt["wpvt"][c][:],
                             start=(c == 0), stop=(c == NCH - 1))
        nc.vector.tensor_tensor(
            out=z[tch][:, 0:D].rearrange("p (h d) -> p h d", h=H),
            in0=vp[:].rearrange("p (h d) -> p h d", h=H),
            in1=e[:].broadcast_to([128, H, HD]),
            op=OP.mult)
        nc.vector.tensor_copy(z[tch][:, D:ZF], e[:])

    czi = dpool.tile([TPC, ZF], F16, tag="czi", name="czi")
    czo = dpool.tile([L, ZF], F16, tag="czo", name="czo")
    for t in range(4):
        dma(czi[t * 128:(t + 1) * 128, :], z[t][:])
    nc.gpsimd.collective_compute(
        "AllGather", OP.bypass, replica_groups=pairs,
        ins=[czi.opt()], outs=[czo.opt()])

    # ---- boundary chain continues (fp32r MLP; cos in true fp32) ----
    v = [apool.tile([128, TPC], FP, tag=f"v_{c}", name=f"v_{c}")
         for c in range(NCH)]
    for g, (g0, gn) in enumerate(TG):
        for ech in range(NCH):
            ps = linear_fm("w2t", t1, ech, g, resid=True)
            nc.vector.tensor_scalar(out=v[ech][:, g0:g0 + gn], in0=ps,
                                    scalar1=col(bias2, ech), scalar2=None,
                                    op0=OP.add)

    u = [apool.tile([128, TPC], FR, tag=f"u_{c}", name=f"u_{c}")
         for c in range(NCH)]
    l2norm_fm(v, u)

    qh = [apool.tile([128, TPC], FP, tag=f"qh_{c}", name=f"qh_{c}")
          for c in range(NCH)]
    kh = [apool.tile([128, TPC], FP, tag=f"kh_{c}", name=f"kh_{c}")
          for c in range(NCH)]
    for g, (g0, gn) in enumerate(TG):
        for dst, w in ((qh, "wqt"), (kh, "wkt")):
            for ech in range(NCH):
                ps = linear_fm(w, u, ech, g)
                nc.scalar.copy(dst[ech][:, g0:g0 + gn], ps)

    # stage boundary kh/qh columns into one tile -> one exchange DMA
    stag = rpool.tile([128, 8], FP, tag="stag", name="stag")
    for c in range(NCH):
        nc.vector.tensor_copy(stag[:, c:c + 1], kh[c][:, 0:1])
        nc.vector.tensor_copy(stag[:, 4 + c:5 + c], qh[c][:, TPC - 1:TPC])

    cosps = ps_small()
    for c in range(NCH):
        pr = spool.tile([128, TPC], FP, tag="prod", name="prod")
        nc.vector.tensor_mul(pr[:, 0:TPC - 1], qh[c][:, 0:TPC - 1],
                             kh[c][:, 1:TPC])
        nc.vector.memset(pr[:, TPC - 1:TPC], 0.0)
        nc.tensor.matmul(cosps[0:1, :], ones128[:], pr[:],
                         start=(c == 0), stop=(c == NCH - 1))
    sgn = rpool.tile([1, TPC], FP, tag="sgn", name="sgn")[0:1, :]
    nc.scalar.activation(sgn, cosps[0:1, :], AF.Sign, bias=simb[0:1, 0:1])
    hard = rpool.tile([1, TPC], FP, tag="hard", name="hard")[0:1, :]
    nc.scalar.activation(hard, sgn, AF.Relu, scale=-1.0)

    # ---- exchange 1: hard row + boundary kh/qh columns, pairwise ----
    cc1i = dpool.tile([1, CCW], FP, tag="cc1i", name="cc1i")
    cc1o = dpool.tile([2, CCW], FP, tag="cc1o", name="cc1o")
    dma(cc1i[0:1, TPC:CCW].rearrange("a (c p) -> (a p) c", p=128), stag[:])
    nc.sync.dma_start(out=cc1i[0:1, 0:TPC], in_=hard)
    nc.gpsimd.collective_compute(
        "AllGather", OP.bypass, replica_groups=pairs,
        ins=[cc1i.opt()], outs=[cc1o.opt()])

    # z blocks back to SBUF: 8 DMAs spread across queues, issued after
    # the cc1 trigger so its tiny DMAs see idle queues
    zf = [None] * 8
    for k in range(8):
        tag = f"z_{k}" if k < 4 else f"hn_{k - 4}"
        zk = apool.tile([128, ZF], F16, tag=tag, name=f"zf_{k}")
        dma(zk[:], czo[k * 128:(k + 1) * 128, :])
        zf[k] = zk

    # straddle pair (global 511, 512): qh_last of rank0 . kh_first of rank1
    strq = rpool.tile([1, TPC], FP, tag="strq", name="strq")[0:1, :]
    dma(strq, cc1o[0:1, 2 * TPC:3 * TPC])
    strk = rpool.tile([1, TPC], FP, tag="strk", name="strk")[0:1, :]
    dma(strk, cc1o[1:2, TPC:2 * TPC])
    nc.vector.tensor_mul(strq, strq, strk)
    scos = rpool.tile([1, 1], FP, tag="scos", name="scos")
    nc.vector.reduce_sum(scos[:], strq, axis=AX.X)
    nc.scalar.activation(scos[:], scos[:], AF.Sign, bias=simb[0:1, 0:1])
    sdel = rpool.tile([1, 1], FP, tag="sdel", name="sdel")
    nc.scalar.activation(sdel[:], scos[:], AF.Relu, scale=-1.0)
    # delta = straddle_hard * vm[511] * (1 - setm[511])  (host const)
    nc.vector.tensor_scalar(out=sdel[:], in0=sdel[:],
                            scalar1=strc[0:1, 0:1], scalar2=None,
                            op0=OP.mult)

    # ---- seg ids, entirely in [128, 8] token-chunk layout ----
    hbt = rpool.tile([128, 8], FP, tag="hbt", name="hbt")
    dma(hbt[:, 0:4],
        cc1o[0:1, 0:TPC].rearrange("a (c p) -> (a p) c", p=128))
    dma(hbt[:, 4:8],
        cc1o[1:2, 0:TPC].rearrange("a (c p) -> (a p) c", p=128))
    nc.vector.tensor_mul(hbt[:], hbt[:], vm8[:])
    nc.vector.tensor_max(hbt[:], hbt[:], st8[:])
    totp = ps_small()
    nc.tensor.matmul(totp[0:1, 0:8], ones128[:], hbt[:],
                     start=True, stop=True)
    tot = rpool.tile([1, 8], FP, tag="tot", name="tot")
    nc.vector.tensor_copy(tot[:], totp[0:1, 0:8])
    offs = rpool.tile([1, 8], FP, tag="offs", name="offs")
    nc.vector.tensor_tensor_scan(offs[:], tot[:], tot[:], 0.0,
                                 OP.add, OP.bypass)
    nc.vector.tensor_sub(offs[:], offs[:], tot[:])      # exclusive prefix
    nc.vector.tensor_scalar(out=offs[0:1, 4:8], in0=offs[0:1, 4:8],
                            scalar1=sdel[0:1, 0:1], scalar2=None,
                            op0=OP.add)
    segp = ps_small()
    nc.tensor.matmul(segp[:, 0:8], ltri[:], hbt[:],
                     start=True, stop=False)
    nc.tensor.matmul(segp[:, 0:8], ones_r[:], offs[:],
                     start=False, stop=True)
    sego = rpool.tile([128, 8], FP, tag="sego", name="sego")
    nc.vector.tensor_copy(sego[:], segp[:, 0:8])

    # ---- pooling into own s-half + normalize, fused per s-chunk ----
    Af = [apool.tile([128, TPC], F16,
                     tag=f"xT{k}" if k < 4 else f"t1_{k - 4}",
                     name=f"Af_{k}") for k in range(8)]
    for k in range(8):
        nc.vector.tensor_scalar(out=Af[k][:], in0=iota512[:],
                                scalar1=sego[:, k:k + 1], scalar2=None,
                                op0=OP.is_equal)
    # denominators for ALL slots upfront (overlaps the psA matmuls):
    # psBall[h, s] = sum_k e[k]^T.Af[k]; recs_c[f, s] = 1/den[head(f), s]
    psBall = ps_small()
    for k in range(8):
        nc.tensor.matmul(psBall[0:H, :], zf[k][:, D:ZF], Af[k][:],
                         start=(k == 0), stop=(k == 7))
    dd = spool.tile([8, TPC], FP, tag="dd", name="dd")
    nc.vector.tensor_scalar(out=dd[:], in0=psBall[0:H, :], scalar1=0.0,
                            scalar2=None, op0=OP.is_equal)
    nc.vector.tensor_add(dd[:], dd[:], psBall[0:H, :])
    rec = spool.tile([8, TPC], FP, tag="rec", name="rec")
    nc.vector.reciprocal_approx_fast(rec[:], dd[:])
    recs = [apool.tile([128, TPC], FP, tag=f"v_{c}", name=f"recs_{c}")
            for c in range(NCH)]
    for c in range(NCH):
        psR = ps_small()
        nc.tensor.matmul(psR[:], e8[:, c * 128:(c + 1) * 128], rec[:],
                         start=True, stop=True)
        nc.scalar.copy(recs[c][:], psR[:])

    # swapped orientation: psA[feat, slot] = zf^T . Af per feat chunk;
    # Wpo applied directly on the normalized [feat, slot] tile (no PE
    # transposes).
    for j in range(4):
        jsl = slice(j * 128, (j + 1) * 128)
        ks = list(range(j, 8))
        psA = ps_big()
        for i, k in enumerate(ks):
            for c in range(NCH):
                nc.tensor.matmul(psA[:, c * 128:(c + 1) * 128],
                                 zf[k][:, c * 128:(c + 1) * 128],
                                 Af[k][:, jsl],
                                 start=(i == 0 and c == 0),
                                 stop=(i == len(ks) - 1 and c == NCH - 1))
        pn = spool.tile([128, D], F16, tag="pn", name="pn")
        for c in range(NCH):
            nc.vector.tensor_mul(pn[:, c * 128:(c + 1) * 128],
                                 psA[:, c * 128:(c + 1) * 128],
                                 recs[c][:, jsl])
        psO = ps_small()
        for c in range(NCH):
            nc.tensor.matmul(psO[:], pn[:, c * 128:(c + 1) * 128],
                             wt["wpot"][c][:],
                             start=(c == 0), stop=(c == NCH - 1))
        osb = spool.tile([128, D], FP, tag="osb", name="osb")
        nc.vector.tensor_copy(osb[:], psO[:])
        dma(out[j * 128:(j + 1) * 128, :], osb[:])

    ctx.close()


def _build():
    if "nc" in _CACHE:
        return _CACHE["nc"]
    nc = bacc.Bacc("TRN2", target_bir_lowering=False, debug=False,
                   num_devices=N_CORES)
    names = {
        "hT": (D, TPC), "w1t": (D, D), "w2t": (D, D), "wqt": (D, D),
        "wkt": (D, D), "wpvt": (D, D), "wpot": (D, D),
        "ones128": (128, 1), "ones_r": (1, 128), "ident": (128, 128),
        "iota512": (128, TPC), "simb": (1, 1),
        "vm8": (128, 8), "st8": (128, 8), "strc": (1, 1),
        "ltri": (128, 128), "lenmask": (128, 4), "e8": (8, D),
        "wkeff": (D, H), "bias1": (128, NCH), "bias2": (128, NCH),
        "lng": (128, NCH), "lnb": (128, NCH),
    }
    _fr = {"w1t", "w2t", "wqt", "wkt", "wpvt", "wkeff"}
    prm = {}
    for k, sh in names.items():
        dt = FR if k in _fr else (F16 if k == "wpot" else FP)
        prm[k] = nc.dram_tensor(k, list(sh), dt, kind="ExternalInput").ap()
    out = nc.dram_tensor("out", [TPC, D], FP, kind="ExternalOutput").ap()
    with tile.TileContext(nc) as tc:
        _emit(nc, tc, prm, out)
    nc.compile()
    _CACHE["nc"] = nc
    return nc


def _host_prep(inputs):
    f32 = np.float32
    f64 = np.float64
    hidden = np.asarray(inputs["hidden"], f32)
    lengths = np.asarray(inputs["lengths"], f32)
    consts = {
        "ones128": np.ones((128, 1), f32),
        "ones_r": np.ones((1, 128), f32),
        "ident": np.eye(128, dtype=f32),
        "ltri": np.triu(np.ones((128, 128), f32), 1),
        "simb": np.asarray(inputs["sim_bias"], f32).reshape(1, 1),
        "wkeff": np.ascontiguousarray(
            (np.asarray(inputs["Wpk"], f64).T.reshape(D, H, HD)
             * np.asarray(inputs["learned_query"],
                          f64).reshape(H, HD)[None]
             ).sum(-1).astype(f32)),
        "bias1": np.ascontiguousarray(
            np.asarray(inputs["b1"], f32).reshape(NCH, 128).T),
        "bias2": np.ascontiguousarray(
            np.asarray(inputs["b2"], f32).reshape(NCH, 128).T),
        "lng": np.ascontiguousarray(
            np.asarray(inputs["ln_g"], f32).reshape(NCH, 128).T),
        "lnb": np.ascontiguousarray(
            np.asarray(inputs["ln_b"], f32).reshape(NCH, 128).T),
    }
    for k, w in (("w1t", "W1"), ("w2t", "W2"), ("wqt", "Wq"), ("wkt", "Wk"),
                 ("wpvt", "Wpv")):
        consts[k] = np.ascontiguousarray(np.asarray(inputs[w], f32).T)
    consts["wpot"] = np.ascontiguousarray(
        np.asarray(inputs["Wpo"], np.float16).T)
    consts["e8"] = np.ascontiguousarray(
        (np.arange(D)[None, :] // HD == np.arange(H)[:, None]).astype(f32))

    actual = (lengths * f32(L + 1)).astype(np.int32)
    valid = np.clip(actual - 1, 0, L)
    cut = (lengths * f32(L)).astype(np.int32)
    grid = np.arange(L).reshape(8, 128).T        # [p, c] -> token c*128+p

    in_maps = []
    for c in range(N_CORES):
        b, h = c // 2, c % 2
        tok0 = h * TPC
        hT = np.ascontiguousarray(hidden[b, tok0:tok0 + TPC, :].T)
        vm8 = ((grid < valid[b]) & (grid < L - 1)).astype(f32)
        st8 = np.zeros((128, 8), f32)
        if valid[b] < L:
            st8[valid[b] % 128, valid[b] // 128] = 1.0
        stv = 1.0 if valid[b] == 511 else 0.0
        strc = np.full((1, 1), (1.0 if 511 < valid[b] else 0.0) * (1 - stv),
                       f32)
        lm = np.zeros((128, 4), f32)
        for tch in range(4):
            g = tok0 + tch * 128 + np.arange(128)
            lm[:, tch] = (g < cut[b]).astype(f32)
        m = dict(consts)
        m.update({
            "hT": hT, "lenmask": lm, "vm8": vm8, "st8": st8, "strc": strc,
            "iota512": np.tile(np.arange(TPC, dtype=f32) + 512.0 * h,
                               (128, 1)),
        })
        in_maps.append(m)
    return in_maps


def kernel(**inputs):
    nc = _build()
    in_maps = _host_prep(inputs)
    res = run_bass_kernel_spmd(nc, in_maps, list(range(N_CORES)))
    out = np.empty((B, L, D), np.float32)
    for c in range(N_CORES):
        b, h = c // 2, c % 2
        out[b, h * TPC:(h + 1) * TPC, :] = res.results[c]["out"]
    return out


# revision 23
# speedup vs baseline: 1.0096x; 1.0096x over previous
"""Trainium2 Bass kernel for BoundaryPredictor2 (B=4, L=1024, D=512, H=8).

Sharding: 8 cores = 4 batch rows x 2 token-halves (512 tokens each).

Phase C-first (feature-major activations throughout, zero PE transposes):
  l2norm -> layernorm -> z' matmul + fused scores (host-precomputed
  Wpk^T@q and Wpo@Wpv) -> z' = (hn@Weff^T)*e; ONE merged pairwise
  AllGather of z' (532KB) fires early and hides under the boundary chain.
Phase A (boundary chain, fp32r matmuls; cos reduction in true fp32):
  MLP(gelu) -> residual l2norm -> qh/kh -> adjacent cos -> hard.
  The cross-half "straddle" pair ships through the hard-AllGather and is
  patched into the chunk offsets only (it provably can't change any
  other seg id).
Phase B: seg ids entirely in [128, 8] token-chunk layout: strictly-upper
  triangular matmul gives the within-chunk exclusive prefix, chunk
  totals via ones-reduction, an 8-wide scan for chunk offsets, both
  accumulated in one PSUM bank. No [1, 1024] scans, no DRAM round trip.
Pooling: one-hot A blocks x z' -> own 512 segment slots in PSUM
  (Wpo pre-folded, so PSUM rows are final up to 1/denom), normalize,
  store [512, 512].

The program is uniform SPMD; all per-core differences (token slice,
masks, iota offset) are host-fed data. A dummy warmup collective
triggers at ~2us to absorb the CC engine's ~40us init latency.
"""

from contextlib import ExitStack

import numpy as np

import concourse.bass as bass
import concourse.tile as tile
from concourse import bacc, mybir
from concourse.bass_utils import run_bass_kernel_spmd

FP = mybir.dt.float32
FR = mybir.dt.float32r
F16 = mybir.dt.float16
AF = mybir.ActivationFunctionType
OP = mybir.AluOpType
AX = mybir.AxisListType

N_CORES = 8
B, L, D = 4, 1024, 512
H, HD = 8, 64
TPC = 512           # own tokens per core
NCH = D // 128      # feature chunks (4)
SCALE = HD ** -0.5
ZF = D + H          # z feature width (512 folded-V cols + 8 e cols)
CCW = 3 * TPC       # exchange-1 payload: hard row + kh col0 + qh col511
TG = ((0, 256), (256, 256))   # phase-A token pipeline groups

_CACHE = {}


def _emit(nc, tc, prm, out):
    ctx = ExitStack()
    cpool = ctx.enter_context(tc.tile_pool(name="consts", bufs=1))
    wpool = ctx.enter_context(tc.tile_pool(name="weights", bufs=1))
    apool = ctx.enter_context(tc.tile_pool(name="acts", bufs=1))
    spool = ctx.enter_context(tc.tile_pool(name="scratch", bufs=2))
    rpool = ctx.enter_context(tc.tile_pool(name="rows", bufs=1))
    psm = ctx.enter_context(tc.tile_pool(name="psm", bufs=3, space="PSUM"))
    pbig = ctx.enter_context(tc.tile_pool(name="pbig", bufs=4, space="PSUM"))
    dpool = ctx.enter_context(tc.tile_pool(name="dram", bufs=1, space="DRAM"))

    def ps_small():
        return psm.tile([128, 512], FP, tag="ps1", name="ps1")

    def ps_big():
        return pbig.tile([128, 512], FP, tag="pb", name="pb")

    def dma(dst, src):
        nc.sync.dma_start(out=dst, in_=src)

    def load(pool, name, shape, dt=FP, tag=None):
        t = pool.tile(list(shape), dt, tag=tag or name, name=name)
        dma(t[:], prm[name])
        return t

    # ---- warmup collective FIRST: absorbs the CC engine init latency ----
    pairs = [[2 * i, 2 * i + 1] for i in range(N_CORES // 2)]
    simb = load(cpool, "simb", (1, 1))
    wui = dpool.tile([1, 1], FP, tag="wui", name="wui")
    wuo = dpool.tile([2, 1], FP, tag="wuo", name="wuo")
    nc.sync.dma_start(out=wui[:], in_=simb[:])
    nc.gpsimd.collective_compute(
        "AllGather", OP.bypass, replica_groups=pairs,
        ins=[wui.opt()], outs=[wuo.opt()])

    # ---- input DMAs in consumption order ----
    xT = [apool.tile([128, TPC], FP, tag=f"xT{c}", name=f"xT{c}")
          for c in range(NCH)]
    for c in range(NCH):
        dma(xT[c][:], prm["hT"][c * 128:(c + 1) * 128, :])
    ones128 = load(cpool, "ones128", (128, 1))
    ones_r = load(cpool, "ones_r", (1, 128))
    ident = load(cpool, "ident", (128, 128))
    wkeff = cpool.tile([128, NCH * H], FR, tag="wkeff", name="wkeff")
    for c in range(NCH):
        dma(wkeff[:, c * H:(c + 1) * H],
            prm["wkeff"][c * 128:(c + 1) * 128, :])
    lenmask = load(cpool, "lenmask", (128, 4))
    lng = load(cpool, "lng", (128, NCH))
    lnb = load(cpool, "lnb", (128, NCH))
    bias1 = load(cpool, "bias1", (128, NCH))
    bias2 = load(cpool, "bias2", (128, NCH))

    wt = {}
    for w in ("wpvt", "w1t", "w2t", "wqt", "wkt", "wpot"):
        wdt = F16 if w == "wpot" else FR
        wt[w] = [wpool.tile([128, D], wdt, tag=f"{w}{c}", name=f"{w}{c}")
                 for c in range(NCH)]
        for c in range(NCH):
            dma(wt[w][c][:], prm[w][c * 128:(c + 1) * 128, :])
    e8 = load(cpool, "e8", (8, D))

    vm8 = load(cpool, "vm8", (128, 8))
    st8 = load(cpool, "st8", (128, 8))
    strc = load(cpool, "strc", (1, 1))
    ltri = load(cpool, "ltri", (128, 128))
    iota512 = load(cpool, "iota512", (128, TPC))

    # FR twins (fp32r matmul operands must be produced as fp32r)
    ones128r = cpool.tile([128, 1], FR, tag="ones128r", name="ones128r")
    nc.vector.tensor_copy(ones128r[:], ones128[:])
    ones_rr = cpool.tile([1, 128], FR, tag="ones_rr", name="ones_rr")
    nc.vector.tensor_copy(ones_rr[:], ones_r[:])
    identr = cpool.tile([128, 128], FR, tag="identr", name="identr")
    nc.vector.tensor_copy(identr[:], ident[:])

    def col(t, c):
        return t[:, c:c + 1]

    # 1/sqrt via ACT Sqrt (<=2ulp) + fast DVE reciprocal (~51ulp):
    # norm-scale errors are multiplicative on cos, so sign-safe.
    def rsqrt_row(dst, src, eps, mode, fr=False, w=TPC):
        sm = rpool.tile([1, TPC], FP, tag="rs_sm", name="rs_sm",
                        bufs=3)[0:1, 0:w]
        nc.vector.tensor_scalar(out=sm, in0=src, scalar1=eps,
                                scalar2=None,
                                op0=(OP.max if mode == "clip" else OP.add))
        sqv = rpool.tile([1, TPC], FP, tag="rs_sq", name="rs_sq",
                         bufs=3)[0:1, 0:w]
        nc.scalar.activation(sqv, sm, AF.Sqrt)
        if fr:
            r0 = rpool.tile([1, TPC], FP, tag="rs_r0", name="rs_r0",
                            bufs=3)[0:1, 0:w]
            nc.vector.reciprocal_approx_fast(r0, sqv)
            nc.vector.tensor_copy(dst, r0)
        else:
            nc.vector.reciprocal_approx_fast(dst, sqv)

    # dst[c] = src[c] * rsqrt(sum_d src^2); token-group pipelined
    def l2norm_fm(src_tiles, dst_tiles, msq_keep=None):
        pss = {}
        for g, (g0, gn) in enumerate(TG):
            ps = ps_small()
            for c in range(NCH):
                sq = spool.tile([128, 256], FR, tag="sq", name="sq", bufs=4)
                nc.vector.tensor_mul(sq[:], src_tiles[c][:, g0:g0 + gn],
                                     src_tiles[c][:, g0:g0 + gn])
                nc.tensor.matmul(ps[0:1, 0:gn], ones128r[:], sq[:],
                                 start=(c == 0), stop=(c == NCH - 1))
            pss[g] = ps
            r = rpool.tile([1, 256], FR, tag="nrm_r", name="nrm_r", bufs=2)
            rsqrt_row(r[0:1, :], ps[0:1, 0:gn], 1e-16, "clip", fr=True,
                      w=gn)
            rb = ps_big()
            nc.tensor.matmul(rb[:, 0:gn], ones_rr[:], r[0:1, :],
                             start=True, stop=True)
            for c in range(NCH):
                nc.vector.tensor_mul(dst_tiles[c][:, g0:g0 + gn],
                                     src_tiles[c][:, g0:g0 + gn],
                                     rb[:, 0:gn])
        if msq_keep is not None:
            nc.vector.tensor_copy(msq_keep[0:1, 0:256], pss[0][0:1, 0:256])
            nc.vector.tensor_copy(msq_keep[0:1, 256:512], pss[1][0:1, 0:256])

    # ---- l2norm of hidden (shared by boundary chain and layernorm) ----
    hn2r = [apool.tile([128, TPC], FR, tag=f"hn2r_{c}", name=f"hn2r_{c}")
            for c in range(NCH)]
    msq = rpool.tile([1, TPC], FP, tag="msq", name="msq")[0:1, :]
    l2norm_fm(xT, hn2r, msq_keep=msq)

    # ---- boundary-chain linears (emitted interleaved with LN/z so the
    # PE stays busy through the LN stats serial chain) ----
    def linear_fm(w, src_tiles, ech, g, resid=False):
        g0, gn = TG[g]
        ps = ps_big()
        for c in range(NCH):
            nc.tensor.matmul(ps[:, 0:gn],
                             wt[w][c][:, ech * 128:(ech + 1) * 128],
                             src_tiles[c][:, g0:g0 + gn],
                             start=(c == 0), stop=False if resid else
                             (c == NCH - 1))
        if resid:
            nc.tensor.matmul(ps[:, 0:gn], identr[:],
                             hn2r[ech][:, g0:g0 + gn],
                             start=False, stop=True)
        return ps[:, 0:gn]

    t1 = [apool.tile([128, TPC], FR, tag=f"t1_{c}", name=f"t1_{c}")
          for c in range(NCH)]

    def w1_pass(g):
        g0, gn = TG[g]
        for ech in range(NCH):
            ps = linear_fm("w1t", hn2r, ech, g)
            nc.scalar.activation(t1[ech][:, g0:g0 + gn], ps, AF.Gelu,
                                 bias=col(bias1, ech))

    w1_pass(0)

    # ---- layernorm -> z' + scores; merged early z AllGather ----
    mups = ps_small()
    for c in range(NCH):
        nc.tensor.matmul(mups[0:1, :], ones128[:], xT[c][:],
                         start=(c == 0), stop=(c == NCH - 1))
    mu = rpool.tile([1, TPC], FP, tag="mu", name="mu")[0:1, :]
    nc.vector.tensor_scalar(out=mu, in0=mups[0:1, :], scalar1=1.0 / D,
                            scalar2=None, op0=OP.mult)
    var = rpool.tile([1, TPC], FP, tag="var", name="var")[0:1, :]
    nc.vector.tensor_scalar(out=var, in0=msq, scalar1=1.0 / D,
                            scalar2=None, op0=OP.mult)
    mu2 = rpool.tile([1, TPC], FP, tag="mu2", name="mu2")[0:1, :]
    nc.vector.tensor_mul(mu2, mu, mu)
    nc.vector.tensor_sub(var, var, mu2)
    rstd = rpool.tile([1, TPC], FP, tag="rstd", name="rstd")[0:1, :]
    rsqrt_row(rstd, var, 1e-5, "add")
    mub = ps_big()
    nc.tensor.matmul(mub[:], ones_r[:], mu, start=True, stop=True)
    rstdb = ps_big()
    nc.tensor.matmul(rstdb[:], ones_r[:], rstd, start=True, stop=True)
    w1_pass(1)
    hn = [apool.tile([128, TPC], FR, tag=f"hn_{c}", name=f"hn_{c}")
          for c in range(NCH)]
    for c in range(NCH):
        ht = spool.tile([128, TPC], FP, tag="htmp", name="htmp")
        nc.vector.tensor_sub(ht[:], xT[c][:], mub[:])
        nc.vector.tensor_mul(ht[:], ht[:], rstdb[:])
        nc.vector.tensor_scalar(out=hn[c][:], in0=ht[:],
                                scalar1=col(lng, c), scalar2=col(lnb, c),
                                op0=OP.mult, op1=OP.add)

    z = [apool.tile([128, ZF], F16, tag=f"z_{t}", name=f"z_{t}")
         for t in range(4)]
    for tch in range(4):
        tsl = slice(tch * 128, (tch + 1) * 128)
        scps = ps_small()
        for c in range(NCH):
            nc.tensor.matmul(scps[:, 0:H], hn[c][:, tsl],
                             wkeff[:, c * H:(c + 1) * H],
                             start=(c == 0), stop=(c == NCH - 1))
        e = spool.tile([128, H], FP, tag="e", name="e")
        nc.scalar.activation(e[:], scps[:, 0:H], AF.Exp, scale=SCALE)
        nc.vector.tensor_scalar(out=e[:], in0=e[:],
                                scalar1=lenmask[:, tch:tch + 1], scalar2=None,
                                op0=OP.mult)
        vp = ps_big()
        for c in range(NCH):
            nc.tensor.matmul(vp[:], hn[c][:, tsl], wt["wpvt"][c][:],
                             start=(c == 0), stop=(c == NCH - 1))
        nc.vector.tensor_tensor(
            out=z[tch][:, 0:D].rearrange("p (h d) -> p h d", h=H),
            in0=vp[:].rearrange("p (h d) -> p h d", h=H),
            in1=e[:].broadcast_to([128, H, HD]),
            op=OP.mult)
        nc.vector.tensor_copy(z[tch][:, D:ZF], e[:])

    czi = dpool.tile([TPC, ZF], F16, tag="czi", name="czi")
    czo = dpool.tile([L, ZF], F16, tag="czo", name="czo")
    for t in range(4):
        dma(czi[t * 128:(t + 1) * 128, :], z[t][:])
    nc.gpsimd.collective_compute(
        "AllGather", OP.bypass, replica_groups=pairs,
        ins=[czi.opt()], outs=[czo.opt()])

    # ---- boundary chain continues (fp32r MLP; cos in true fp32) ----
    v = [apool.tile([128, TPC], FP, tag=f"v_{c}", name=f"v_{c}")
         for c in range(NCH)]
    for g, (g0, gn) in enumerate(TG):
        for ech in range(NCH):
            ps = linear_fm("w2t", t1, ech, g, resid=True)
            nc.vector.tensor_scalar(out=v[ech][:, g0:g0 + gn], in0=ps,
                                    scalar1=col(bias2, ech), scalar2=None,
                                    op0=OP.add)

    u = [apool.tile([128, TPC], FR, tag=f"u_{c}", name=f"u_{c}")
         for c in range(NCH)]
    l2norm_fm(v, u)

    qh = [apool.tile([128, TPC], FP, tag=f"qh_{c}", name=f"qh_{c}")
          for c in range(NCH)]
    kh = [apool.tile([128, TPC], FP, tag=f"kh_{c}", name=f"kh_{c}")
          for c in range(NCH)]
    # stage boundary kh/qh columns into one tile -> one exchange DMA;
    # copies are emitted as soon as their source group lands
    stag = rpool.tile([128, 8], FP, tag="stag", name="stag")
    for g, (g0, gn) in enumerate(TG):
        for dst, w in ((qh, "wqt"), (kh, "wkt")):
            for ech in range(NCH):
                ps = linear_fm(w, u, ech, g)
                nc.scalar.copy(dst[ech][:, g0:g0 + gn], ps)
                if g == 0 and w == "wkt":
                    nc.vector.tensor_copy(stag[:, ech:ech + 1],
                                          kh[ech][:, 0:1])
                if g == 1 and w == "wqt":
                    nc.vector.tensor_copy(stag[:, 4 + ech:5 + ech],
                                          qh[ech][:, TPC - 1:TPC])

    cosps = ps_small()
    for c in range(NCH):
        pr = spool.tile([128, TPC], FP, tag="prod", name="prod")
        nc.vector.tensor_mul(pr[:, 0:TPC - 1], qh[c][:, 0:TPC - 1],
                             kh[c][:, 1:TPC])
        nc.vector.memset(pr[:, TPC - 1:TPC], 0.0)
        nc.tensor.matmul(cosps[0:1, :], ones128[:], pr[:],
                         start=(c == 0), stop=(c == NCH - 1))
    sgn = rpool.tile([1, TPC], FP, tag="sgn", name="sgn")[0:1, :]
    nc.scalar.activation(sgn, cosps[0:1, :], AF.Sign, bias=simb[0:1, 0:1])
    hard = rpool.tile([1, TPC], FP, tag="hard", name="hard")[0:1, :]
    nc.scalar.activation(hard, sgn, AF.Relu, scale=-1.0)

    # ---- exchange 1: hard row + boundary kh/qh columns, pairwise ----
    cc1i = dpool.tile([1, CCW], FP, tag="cc1i", name="cc1i")
    cc1o = dpool.tile([2, CCW], FP, tag="cc1o", name="cc1o")
    dma(cc1i[0:1, TPC:CCW].rearrange("a (c p) -> (a p) c", p=128), stag[:])
    nc.sync.dma_start(out=cc1i[0:1, 0:TPC], in_=hard)
    nc.gpsimd.collective_compute(
        "AllGather", OP.bypass, replica_groups=pairs,
        ins=[cc1i.opt()], outs=[cc1o.opt()])

    # Global blocks 0-3 pool straight from the local z tiles: for even
    # cores they ARE blocks 0-3; for odd cores blocks 0-3 can never land
    # in the owned slot range, so the one-hots are all-zero and the
    # (wrong) values are masked. Only the second czo half is downloaded.
    zpeer = apool.tile([128, 4 * ZF], F16, tag="zpeer", name="zpeer")
    dma(zpeer[:].rearrange("p (b f) -> p b f", b=4),
        czo[TPC:L, :].rearrange("(b p) f -> p b f", p=128))
    zf = z + [zpeer[:, k * ZF:(k + 1) * ZF] for k in range(4)]

    # straddle pair (global 511, 512): qh_last of rank0 . kh_first of rank1
    strq = rpool.tile([1, TPC], FP, tag="strq", name="strq")[0:1, :]
    dma(strq, cc1o[0:1, 2 * TPC:3 * TPC])
    strk = rpool.tile([1, TPC], FP, tag="strk", name="strk")[0:1, :]
    dma(strk, cc1o[1:2, TPC:2 * TPC])
    nc.vector.tensor_mul(strq, strq, strk)
    scos = rpool.tile([1, 1], FP, tag="scos", name="scos")
    nc.vector.reduce_sum(scos[:], strq, axis=AX.X)
    nc.scalar.activation(scos[:], scos[:], AF.Sign, bias=simb[0:1, 0:1])
    sdel = rpool.tile([1, 1], FP, tag="sdel", name="sdel")
    nc.scalar.activation(sdel[:], scos[:], AF.Relu, scale=-1.0)
    # delta = straddle_hard * vm[511] * (1 - setm[511])  (host const)
    nc.vector.tensor_scalar(out=sdel[:], in0=sdel[:],
                            scalar1=strc[0:1, 0:1], scalar2=None,
                            op0=OP.mult)

    # ---- seg ids, entirely in [128, 8] token-chunk layout ----
    hbt = rpool.tile([128, 8], FP, tag="hbt", name="hbt")
    dma(hbt[:, 0:4],
        cc1o[0:1, 0:TPC].rearrange("a (c p) -> (a p) c", p=128))
    dma(hbt[:, 4:8],
        cc1o[1:2, 0:TPC].rearrange("a (c p) -> (a p) c", p=128))
    nc.vector.tensor_mul(hbt[:], hbt[:], vm8[:])
    nc.vector.tensor_max(hbt[:], hbt[:], st8[:])
    totp = ps_small()
    nc.tensor.matmul(totp[0:1, 0:8], ones128[:], hbt[:],
                     start=True, stop=True)
    tot = rpool.tile([1, 8], FP, tag="tot", name="tot")
    nc.vector.tensor_copy(tot[:], totp[0:1, 0:8])
    offs = rpool.tile([1, 8], FP, tag="offs", name="offs")
    nc.vector.tensor_tensor_scan(offs[:], tot[:], tot[:], 0.0,
                                 OP.add, OP.bypass)
    nc.vector.tensor_sub(offs[:], offs[:], tot[:])      # exclusive prefix
    nc.vector.tensor_scalar(out=offs[0:1, 4:8], in0=offs[0:1, 4:8],
                            scalar1=sdel[0:1, 0:1], scalar2=None,
                            op0=OP.add)
    segp = ps_small()
    nc.tensor.matmul(segp[:, 0:8], ltri[:], hbt[:],
                     start=True, stop=False)
    nc.tensor.matmul(segp[:, 0:8], ones_r[:], offs[:],
                     start=False, stop=True)
    sego = rpool.tile([128, 8], FP, tag="sego", name="sego")
    nc.vector.tensor_copy(sego[:], segp[:, 0:8])

    # ---- pooling into own s-half + normalize, fused per s-chunk ----
    Af = [apool.tile([128, TPC], F16,
                     tag=f"xT{k}" if k < 4 else f"t1_{k - 4}",
                     name=f"Af_{k}") for k in range(8)]
    for k in range(8):
        nc.vector.tensor_scalar(out=Af[k][:], in0=iota512[:],
                                scalar1=sego[:, k:k + 1], scalar2=None,
                                op0=OP.is_equal)
    # denominators for ALL slots upfront (overlaps the psA matmuls):
    # psBall[h, s] = sum_k e[k]^T.Af[k]; recs_c[f, s] = 1/den[head(f), s]
    psBall = ps_small()
    for k in range(8):
        nc.tensor.matmul(psBall[0:H, :], zf[k][:, D:ZF], Af[k][:],
                         start=(k == 0), stop=(k == 7))
    dd = spool.tile([8, TPC], FP, tag="dd", name="dd")
    nc.vector.tensor_scalar(out=dd[:], in0=psBall[0:H, :], scalar1=0.0,
                            scalar2=None, op0=OP.is_equal)
    nc.vector.tensor_add(dd[:], dd[:], psBall[0:H, :])
    rec = spool.tile([8, TPC], FP, tag="rec", name="rec")
    nc.vector.reciprocal_approx_fast(rec[:], dd[:])
    recs = [apool.tile([128, TPC], FP, tag=f"v_{c}", name=f"recs_{c}")
            for c in range(NCH)]
    for c in range(NCH):
        psR = ps_small()
        nc.tensor.matmul(psR[:], e8[:, c * 128:(c + 1) * 128], rec[:],
                         start=True, stop=True)
        nc.scalar.copy(recs[c][:], psR[:])

    # swapped orientation: psA[feat, slot] = zf^T . Af per feat chunk;
    # Wpo applied directly on the normalized [feat, slot] tile (no PE
    # transposes).
    for j in range(4):
        jsl = slice(j * 128, (j + 1) * 128)
        ks = list(range(j, 8))
        psA = ps_big()
        for i, k in enumerate(ks):
            for c in range(NCH):
                nc.tensor.matmul(psA[:, c * 128:(c + 1) * 128],
                                 zf[k][:, c * 128:(c + 1) * 128],
                                 Af[k][:, jsl],
                                 start=(i == 0 and c == 0),
                                 stop=(i == len(ks) - 1 and c == NCH - 1))
        pn = spool.tile([128, D], F16, tag="pn", name="pn")
        for c in range(NCH):
            nc.vector.tensor_mul(pn[:, c * 128:(c + 1) * 128],
                                 psA[:, c * 128:(c + 1) * 128],
                                 recs[c][:, jsl])
        psO = ps_small()
        for c in range(NCH):
            nc.tensor.matmul(psO[:], pn[:, c * 128:(c + 1) * 128],
                             wt["wpot"][c][:],
                             start=(c == 0), stop=(c == NCH - 1))
        osb = spool.tile([128, D], FP, tag="osb", name="osb")
        nc.vector.tensor_copy(osb[:], psO[:])
        dma(out[j * 128:(j + 1) * 128, :], osb[:])

    ctx.close()


def _build():
    if "nc" in _CACHE:
        return _CACHE["nc"]
    nc = bacc.Bacc("TRN2", target_bir_lowering=False, debug=False,
                   num_devices=N_CORES)
    names = {
        "hT": (D, TPC), "w1t": (D, D), "w2t": (D, D), "wqt": (D, D),
        "wkt": (D, D), "wpvt": (D, D), "wpot": (D, D),
        "ones128": (128, 1), "ones_r": (1, 128), "ident": (128, 128),
        "iota512": (128, TPC), "simb": (1, 1),
        "vm8": (128, 8), "st8": (128, 8), "strc": (1, 1),
        "ltri": (128, 128), "lenmask": (128, 4), "e8": (8, D),
        "wkeff": (D, H), "bias1": (128, NCH), "bias2": (128, NCH),
        "lng": (128, NCH), "lnb": (128, NCH),
    }
    _fr = {"w1t", "w2t", "wqt", "wkt", "wpvt", "wkeff"}
    prm = {}
    for k, sh in names.items():
        dt = FR if k in _fr else (F16 if k == "wpot" else FP)
        prm[k] = nc.dram_tensor(k, list(sh), dt, kind="ExternalInput").ap()
    out = nc.dram_tensor("out", [TPC, D], FP, kind="ExternalOutput").ap()
    with tile.TileContext(nc) as tc:
        _emit(nc, tc, prm, out)
    nc.compile()
    _CACHE["nc"] = nc
    return nc


def _host_prep(inputs):
    f32 = np.float32
    f64 = np.float64
    hidden = np.asarray(inputs["hidden"], f32)
    lengths = np.asarray(inputs["lengths"], f32)
    consts = {
        "ones128": np.ones((128, 1), f32),
        "ones_r": np.ones((1, 128), f32),
        "ident": np.eye(128, dtype=f32),
        "ltri": np.triu(np.ones((128, 128), f32), 1),
        "simb": np.asarray(inputs["sim_bias"], f32).reshape(1, 1),
        "wkeff": np.ascontiguousarray(
            (np.asarray(inputs["Wpk"], f64).T.reshape(D, H, HD)
             * np.asarray(inputs["learned_query"],
                          f64).reshape(H, HD)[None]
             ).sum(-1).astype(f32)),
        "bias1": np.ascontiguousarray(
            np.asarray(inputs["b1"], f32).reshape(NCH, 128).T),
        "bias2": np.ascontiguousarray(
            np.asarray(inputs["b2"], f32).reshape(NCH, 128).T),
        "lng": np.ascontiguousarray(
            np.asarray(inputs["ln_g"], f32).reshape(NCH, 128).T),
        "lnb": np.ascontiguousarray(
            np.asarray(inputs["ln_b"], f32).reshape(NCH, 128).T),
    }
    for k, w in (("w1t", "W1"), ("w2t", "W2"), ("wqt", "Wq"), ("wkt", "Wk"),
                 ("wpvt", "Wpv")):
        consts[k] = np.ascontiguousarray(np.asarray(inputs[w], f32).T)
    consts["wpot"] = np.ascontiguousarray(
        np.asarray(inputs["Wpo"], np.float16).T)
    consts["e8"] = np.ascontiguousarray(
        (np.arange(D)[None, :] // HD == np.arange(H)[:, None]).astype(f32))

    actual = (lengths * f32(L + 1)).astype(np.int32)
    valid = np.clip(actual - 1, 0, L)
    cut = (lengths * f32(L)).astype(np.int32)
    grid = np.arange(L).reshape(8, 128).T        # [p, c] -> token c*128+p

    in_maps = []
    for c in range(N_CORES):
        b, h = c // 2, c % 2
        tok0 = h * TPC
        hT = np.ascontiguousarray(hidden[b, tok0:tok0 + TPC, :].T)
        vm8 = ((grid < valid[b]) & (grid < L - 1)).astype(f32)
        st8 = np.zeros((128, 8), f32)
        if valid[b] < L:
            st8[valid[b] % 128, valid[b] // 128] = 1.0
        stv = 1.0 if valid[b] == 511 else 0.0
        strc = np.full((1, 1), (1.0 if 511 < valid[b] else 0.0) * (1 - stv),
                       f32)
        lm = np.zeros((128, 4), f32)
        for tch in range(4):
            g = tok0 + tch * 128 + np.arange(128)
            lm[:, tch] = (g < cut[b]).astype(f32)
        m = dict(consts)
        m.update({
            "hT": hT, "lenmask": lm, "vm8": vm8, "st8": st8, "strc": strc,
            "iota512": np.tile(np.arange(TPC, dtype=f32) + 512.0 * h,
                               (128, 1)),
        })
        in_maps.append(m)
    return in_maps


def kernel(**inputs):
    nc = _build()
    in_maps = _host_prep(inputs)
    res = run_bass_kernel_spmd(nc, in_maps, list(range(N_CORES)))
    out = np.empty((B, L, D), np.float32)
    for c in range(N_CORES):
        b, h = c // 2, c % 2
        out[b, h * TPC:(h + 1) * TPC, :] = res.results[c]["out"]
    return out
